# revision 1
# baseline (speedup 1.0000x reference)
"""GQA attention kernel for Trainium2, 8 NeuronCores.

Sharding: TP-4 (kv-head pairs) x DP-2 (batch). Core c = b*4 + g handles
batch b, q-heads 8g..8g+7, kv-heads 2g..2g+1. Each core computes a partial
(D, S) output (its heads' contribution through wo); host sums the 4 partials
per batch and transposes.

All device matmuls run in float32r (FP22 multiply, fp32 accumulate) which
streams at 1 cycle/row on the PE when the moving free dim is >= 256.

Layouts are chosen so no on-device transposes are needed:
  - x, weights fed pre-transposed from host (contiguous DMA).
  - QKV projections produce qT/kT (feat, tok); V produced as (tok, feat).
  - scores computed as scores^T (key, query); softmax denominator via
    DVE partial adds + one ones-matmul partition reduction (broadcast form).
  - PV matmul: lhsT = V (tok, vd), rhs = unnormalized probs^T (key, query).
  - normalization folded into the PSUM->SBUF copy.
"""

import sys

if "/opt/trn_rl_repo" not in sys.path:
    sys.path.insert(0, "/opt/trn_rl_repo")

import math
import os

import ml_dtypes
import numpy as np

BF16 = ml_dtypes.bfloat16

B = 2
S = 2048
D = 4096
H = 32
KVH = 8
HD = 128
P = 128
TPG = 4                 # tensor-parallel groups (per batch)
LQH = H // TPG          # 8 local q heads
LKVH = KVH // TPG       # 2 local kv heads
QF = LQH * HD           # 1024 local q features
KF = LKVH * HD          # 256 local kv features
CHUNK = 512
NCHUNK = S // CHUNK     # 4
KT = D // P             # 32 contraction tiles for projections
SCALE = 1.0 / math.sqrt(HD)

_BUILT = None


def _build_program():
    import concourse.bass as bass  # noqa: F401
    import concourse.tile as tile
    from concourse import bacc, mybir

    nc = bacc.Bacc("TRN2", target_bir_lowering=False, debug=False,
                   num_devices=8)
    f32 = mybir.dt.float32
    r32 = mybir.dt.float32r
    b16 = mybir.dt.bfloat16

    xT = nc.dram_tensor("xT", [D, S], b16, kind="ExternalInput").ap()
    wqT = nc.dram_tensor("wqT", [D, QF], b16, kind="ExternalInput").ap()
    wkT = nc.dram_tensor("wkT", [D, KF], b16, kind="ExternalInput").ap()
    wvT = nc.dram_tensor("wvT", [D, KF], b16, kind="ExternalInput").ap()
    woT = nc.dram_tensor("woT", [QF, D], b16, kind="ExternalInput").ap()
    cos2 = nc.dram_tensor("cos2", [HD, S], f32, kind="ExternalInput").ap()
    sinS = nc.dram_tensor("sinS", [HD, S], f32, kind="ExternalInput").ap()
    maskd = nc.dram_tensor("maskd", [P, 4 * CHUNK], f32,
                           kind="ExternalInput").ap()
    outT = nc.dram_tensor("outT", [D, S], b16, kind="ExternalOutput").ap()

    Exp = mybir.ActivationFunctionType.Exp

    with tile.TileContext(nc) as tc:
        with (
            tc.tile_pool(name="consts", bufs=1) as consts,
            tc.tile_pool(name="persist", bufs=1) as persist,
            tc.tile_pool(name="qpool", bufs=1) as qpool,
            tc.tile_pool(name="stream", bufs=1) as stream,
            tc.tile_pool(name="work", bufs=1) as work,
            tc.tile_pool(name="ps", bufs=1, space="PSUM") as ps,
        ):
            # ---- constants (DMAs deferred into chunk 0 pass A) ----
            cos_sb = consts.tile([HD, S], f32, name="cos_sb")
            sin_sb = consts.tile([HD, S], f32, name="sin_sb")
            mask_sb = consts.tile([P, 4, CHUNK], f32, name="mask_sb")
            ones_f = consts.tile([P, P], f32, name="ones_f")
            ones_b = consts.tile([P, P], b16, name="ones_b")

            # ---- persistent K^T (roped) and V ----
            kT_sb = persist.tile([P, LKVH, S], b16, name="kT_sb")
            v_sb = persist.tile([P, S // P, KF], b16, name="v_sb")

            # ---- resident K/V projection weights ----
            wk_sb = persist.tile([P, KT, KF], b16, name="wk_sb")
            wv_sb = persist.tile([P, KT, KF], b16, name="wv_sb")

            def emit_const_loads(kt):
                # interleaved into chunk 0 pass A, a slice per kt iteration
                if kt == 1:
                    nc.sync.dma_start(cos_sb, cos2)
                    nc.sync.dma_start(sin_sb, sinS)
                elif kt == 2:
                    nc.sync.dma_start(mask_sb,
                                      maskd.rearrange("p (r f) -> p r f", r=4))
                    nc.vector.memset(ones_f, 1.0)
                    nc.vector.tensor_copy(out=ones_b, in_=ones_f)
                nc.sync.dma_start(wk_sb[:, kt, :],
                                  wkT[kt * P:(kt + 1) * P, :])
                nc.sync.dma_start(wv_sb[:, kt, :],
                                  wvT[kt * P:(kt + 1) * P, :])

            def psum_tile(nm):
                return ps.tile([P, CHUNK], f32, tag="ps", bufs=8, name=nm)

            def rope(dst, src_psum, tsl, nm):
                """dst = rope(src_psum) for a (128, CHUNK) head tile.

                dst[2i]   = p[2i]*cos_i - p[2i+1]*sin_i
                dst[2i+1] = p[2i]*sin_i + p[2i+1]*cos_i
                cos_sb has cos_i on partitions 2i/2i+1; sin_sb holds -sin_i
                on 2i and +sin_i on 2i+1; swp swaps partition pairs via DMA.
                """
                raw = work.tile([P, CHUNK], f32, tag="rraw", bufs=3,
                                name=f"rw{nm}")
                nc.any.tensor_copy(out=raw, in_=src_psum)
                swp = work.tile([P, CHUNK], f32, tag="rsw", bufs=3,
                                name=f"sw{nm}")
                nc.sync.dma_start(swp[0:P:2, :], raw[1:P:2, :])
                nc.sync.dma_start(swp[1:P:2, :], raw[0:P:2, :])
                nc.vector.tensor_mul(out=dst, in0=src_psum,
                                     in1=cos_sb[:, tsl])
                nc.vector.tensor_mul(out=swp, in0=swp, in1=sin_sb[:, tsl])
                nc.vector.tensor_add(out=dst, in0=dst, in1=swp)

            for qc in range(NCHUNK):
                tsl = slice(qc * CHUNK, (qc + 1) * CHUNK)

                # ======== pass A: Q projection ========
                qT_sb = qpool.tile([P, LQH, CHUNK], b16, tag="qT", bufs=2,
                                   name=f"qT{qc}")
                qps = [psum_tile(f"qp{qc}_{m}") for m in range(LQH)]
                xts = []
                for kt in range(KT):
                    xt = stream.tile([P, CHUNK], b16, tag="xc", bufs=38,
                                     name=f"xa{qc}_{kt}")
                    nc.sync.dma_start(xt, xT[kt * P:(kt + 1) * P, tsl])
                    xts.append(xt)
                    wqt = stream.tile([P, QF], b16, tag="wq", bufs=3,
                                      name=f"wq{qc}_{kt}")
                    nc.sync.dma_start(wqt, wqT[kt * P:(kt + 1) * P, :])
                    if qc == 0:
                        emit_const_loads(kt)
                    for m in range(LQH):
                        nc.tensor.matmul(
                            qps[m],
                            wqt[:, m * P:(m + 1) * P],
                            xt,
                            start=(kt == 0), stop=(kt == KT - 1))
                for m in range(LQH):
                    rope(qT_sb[:, m, :], qps[m], tsl, f"q{qc}_{m}")

                # ======== pass B: K, V projections ========
                kps = [psum_tile(f"kp{qc}_{j}") for j in range(LKVH)]
                vps = [psum_tile(f"vp{qc}_{st}") for st in range(4)]
                for kt in range(KT):
                    xt = xts[kt]
                    for j in range(LKVH):
                        nc.tensor.matmul(
                            kps[j],
                            wk_sb[:, kt, j * P:(j + 1) * P],
                            xt,
                            start=(kt == 0), stop=(kt == KT - 1))
                    for st in range(4):
                        nc.tensor.matmul(
                            vps[st][:, :KF],
                            xt[:, st * P:(st + 1) * P],
                            wv_sb[:, kt, :],
                            start=(kt == 0), stop=(kt == KT - 1))
                for j in range(LKVH):
                    rope(kT_sb[:, j, tsl], kps[j], tsl, f"k{qc}_{j}")
                for st in range(4):
                    nc.vector.tensor_copy(
                        out=v_sb[:, qc * 4 + st, :], in_=vps[st][:, :KF])

                # ======== attention for this chunk ========
                NT = 4 * qc + 4
                LOOK = 3
                flat = [(h, kt) for h in range(LQH) for kt in range(NT)]
                exq = {}
                opvs = {}
                dpss = {}

                def emit_scores(h, kt):
                    kv = h // 4
                    sps = psum_tile(f"sp{qc}_{h}_{kt}")
                    nc.tensor.matmul(
                        sps,
                        kT_sb[:, kv, kt * P:(kt + 1) * P],
                        qT_sb[:, h, :], start=True, stop=True)
                    ex = work.tile([P, CHUNK], b16, tag="exp", bufs=6,
                                   name=f"ex{qc}_{h}_{kt}")
                    nc.scalar.activation(out=ex, in_=sps, func=Exp,
                                         scale=SCALE)
                    r = kt - 4 * qc
                    if r >= 0:
                        nc.vector.tensor_mul(out=ex, in0=ex,
                                             in1=mask_sb[:, r, :])
                    exq[(h, kt)] = ex

                attnT_sb = qpool.tile([P, LQH, CHUNK], b16,
                                      tag="aT", bufs=1, name=f"aT{qc}")
                for h, kt in flat[:LOOK]:
                    emit_scores(h, kt)
                for i, (h, kt) in enumerate(flat):
                    if i + LOOK < len(flat):
                        emit_scores(*flat[i + LOOK])
                    kv = h // 4
                    if kt == 0:
                        opvs[h] = psum_tile(f"ov{qc}_{h}")
                        dpss[h] = psum_tile(f"dp{qc}_{h}")
                    ex = exq.pop((h, kt))
                    nc.tensor.matmul(
                        opvs[h],
                        v_sb[:, kt, kv * P:(kv + 1) * P],
                        ex,
                        start=(kt == 0), stop=(kt == NT - 1))
                    # denominator accumulates on PE: broadcast partition sum
                    nc.tensor.matmul(
                        dpss[h], ones_b, ex,
                        start=(kt == 0), stop=(kt == NT - 1))
                    if kt == NT - 1:
                        drec = work.tile([P, CHUNK], f32, tag="drec", bufs=2,
                                         name=f"dr{qc}_{h}")
                        nc.vector.reciprocal(out=drec, in_=dpss.pop(h))
                        nc.vector.tensor_mul(out=attnT_sb[:, h, :],
                                             in0=opvs.pop(h), in1=drec)

                # ======== output projection ========
                for mtg in range(8):
                    ops = [psum_tile(f"op{qc}_{mtg}_{mi}") for mi in range(4)]
                    for kf in range(LQH):
                        wot = stream.tile([P, CHUNK], b16, tag="wo", bufs=8,
                                          name=f"wo{qc}_{mtg}_{kf}")
                        nc.sync.dma_start(
                            wot, woT[kf * P:(kf + 1) * P,
                                     mtg * CHUNK:(mtg + 1) * CHUNK])
                        for mi in range(4):
                            nc.tensor.matmul(
                                ops[mi],
                                wot[:, mi * P:(mi + 1) * P],
                                attnT_sb[:, kf, :],
                                start=(kf == 0), stop=(kf == LQH - 1))
                    for mi in range(4):
                        osb = work.tile([P, CHUNK], b16, tag="osb", bufs=4,
                                        name=f"ou{qc}_{mtg}_{mi}")
                        nc.vector.tensor_copy(out=osb, in_=ops[mi])
                        mt = mtg * 4 + mi
                        nc.sync.dma_start(
                            outT[mt * P:(mt + 1) * P, tsl], osb)

    nc.compile()
    return nc


def _host_inputs(x, cos, sin, wq, wk, wv, wo):
    """Per-core input dicts. Core c = b*TPG + g."""
    cosT = np.ascontiguousarray(cos.T.astype(np.float32))   # (64, S)
    sinT = np.ascontiguousarray(sin.T.astype(np.float32))
    cos2 = np.repeat(cosT, 2, axis=0)                       # (128, S)
    sinS = np.empty((HD, S), np.float32)
    sinS[0::2] = -sinT
    sinS[1::2] = sinT

    maskd = np.zeros((P, 4, CHUNK), np.float32)
    pp = np.arange(P)[:, None]
    ff = np.arange(CHUNK)[None, :]
    for r in range(4):
        maskd[:, r, :] = (r * P + pp <= ff).astype(np.float32)
    maskd = maskd.reshape(P, 4 * CHUNK)

    in_maps = []
    for c in range(8):
        b, g = divmod(c, TPG)
        qsl = slice(g * QF, (g + 1) * QF)
        ksl = slice(g * KF, (g + 1) * KF)
        in_maps.append({
            "xT": np.ascontiguousarray(x[b].T).astype(BF16),
            "wqT": np.ascontiguousarray(wq[qsl].T).astype(BF16),
            "wkT": np.ascontiguousarray(wk[ksl].T).astype(BF16),
            "wvT": np.ascontiguousarray(wv[ksl].T).astype(BF16),
            "woT": np.ascontiguousarray(wo[:, qsl].T).astype(BF16),
            "cos2": cos2,
            "sinS": sinS,
            "maskd": maskd,
        })
    return in_maps


def kernel(x, cos, sin, wq, wk, wv, wo):
    global _BUILT
    from concourse.bass_utils import run_bass_kernel_spmd

    x = np.asarray(x, np.float32)
    cos = np.asarray(cos, np.float32)
    sin = np.asarray(sin, np.float32)
    wq = np.asarray(wq, np.float32)
    wk = np.asarray(wk, np.float32)
    wv = np.asarray(wv, np.float32)
    wo = np.asarray(wo, np.float32)

    if _BUILT is None:
        _BUILT = _build_program()
    nc = _BUILT

    in_maps = _host_inputs(x, cos, sin, wq, wk, wv, wo)
    trace = os.environ.get("KERNEL_TRACE") == "1"
    res = run_bass_kernel_spmd(nc, in_maps, core_ids=list(range(8)),
                               trace=trace)
    if trace and res.exec_time_ns is not None:
        print(f"HW exec time: {res.exec_time_ns} ns")

    out = np.zeros((B, S, D), np.float32)
    for c in range(8):
        b = c // TPG
        out[b] += res.results[c]["outT"].T.astype(np.float32)
    return out

